# revision 24
# baseline (speedup 1.0000x reference)
"""Multi-head causal attention (B=2, S=2048, D=1024, H=16, DK=DV=64) on 8 Trainium2
NeuronCores.

Sharding: 2-way batch x 4-way head-group. Core i handles batch i//4 and heads
[4*(i%4), 4*(i%4)+4). Each core projects q/k/v for its head group, runs causal
attention, and computes a partial output projection through its row-block of Wo.
The 4 partial outputs per batch are summed on the host.

v2 design (vs the f32r baseline):
- Everything bf16 (inputs downcast on host): full-rate PE, FWL weight loads,
  half the HBM traffic. PSUM accumulation stays fp32.
- Scores for a head PAIR run concurrently on the PE via row-group tiling
  (K=64 each; heads 2mi/2mi+1 live on partitions 0-63/64-127 of qT/kT).
- v is projected in natural [seq, dv] layout (no PE transposes) and scattered
  into padded per-k-tile lhsT blocks: A = [dims(64)|ones|zeros(63)],
  B = [ones|zeros(63)|dims(64)]. attn@v with M=128 then lands head A's
  numerator on PSUM partitions 0-63 (denominator row 64) and head B's on
  64-127 (denominator row 0), so the softmax normalize multiply is a single
  partition-aligned DVE op per head fused with the PSUM->SBUF cast.
- Scores for 2 k-tiles x 2 heads accumulate into one [128, 2048] PSUM region
  (4 banks); exp runs as two [128,1024] ACT instructions (A-half / B-half) so
  the next step's A-scores only wait on the A-half exp (subtile deps).
- Denominator reciprocal via the fast approx DVE op straight out of PSUM;
  broadcast across 64 partitions with a gpsimd partition_broadcast.
- Causal masking: diagonal 128-blocks get a triangular mask multiply on
  gpsimd (SBUF bf16); fully-masked column prefixes are simply skipped by
  restricting the attn@v matmul to [lo:512].
- The whole kernel is software-pipelined with an explicit filler queue:
  projection / normalize / output-projection work is emitted between the
  attention steps so the PE never idles behind the ACT exp chain.
"""
import sys

sys.path.insert(0, "/opt/trn_rl_repo")
import numpy as np

B, S, D = 2, 2048, 1024
H, DK, DV = 16, 64, 64
NCORES = 8
HG = 4          # head-group cores per batch
HPC = 4         # heads per core
HDC = HPC * DK  # 256 projection cols per core
P = 128         # partitions
CH = 512        # q-chunk size
NCH = S // CH   # 4 q-chunks (also 512-col seq blocks)
NST = S // P    # 16 s-tiles (k-tiles)
ND = D // P     # 8 d-tiles
NM = 2          # head pairs


def build(nc, tile, mybir):
    from collections import deque
    from contextlib import ExitStack

    BF16 = mybir.dt.bfloat16
    F32 = mybir.dt.float32
    Exp = mybir.ActivationFunctionType.Exp

    xqT = nc.dram_tensor("xqT", [D, S], BF16, kind="ExternalInput").ap()
    xkT = nc.dram_tensor("xkT", [D, S], BF16, kind="ExternalInput").ap()
    xvT = nc.dram_tensor("xvT", [D, S], BF16, kind="ExternalInput").ap()
    wqkv = nc.dram_tensor("wqkv", [D, 3 * HDC], BF16, kind="ExternalInput").ap()
    wod = nc.dram_tensor("wod", [HDC, D], BF16, kind="ExternalInput").ap()
    maskA = nc.dram_tensor("maskA", [P, P], BF16, kind="ExternalInput").ap()
    vinit = nc.dram_tensor("vinit", [P, 4 * P], BF16, kind="ExternalInput").ap()
    onesr = nc.dram_tensor("onesr", [1, 2 * P], mybir.dt.float32r,
                           kind="ExternalInput").ap()
    out = nc.dram_tensor("out", [S, D], BF16, kind="ExternalOutput").ap()

    with tile.TileContext(nc) as tc:
        with ExitStack() as ctx:
            wp = ctx.enter_context(tc.tile_pool(name="wp", bufs=1))
            ep = ctx.enter_context(tc.tile_pool(name="ep", bufs=2))
            obp = ctx.enter_context(tc.tile_pool(name="obp", bufs=2))
            sp = ctx.enter_context(tc.tile_pool(name="sp", bufs=4))
            rp = ctx.enter_context(tc.tile_pool(name="rp", bufs=2))
            pp_ps = ctx.enter_context(tc.tile_pool(name="pp_ps", bufs=2, space="PSUM"))
            reg_ps = ctx.enter_context(tc.tile_pool(name="reg_ps", bufs=1, space="PSUM"))
            ov_ps = ctx.enter_context(tc.tile_pool(name="ov_ps", bufs=2, space="PSUM"))

            # ---- persistent SBUF tiles ----
            xq_t = [wp.tile([P, S], BF16, name=f"xq{i}") for i in range(ND)]
            xk_t = [wp.tile([P, S], BF16, name=f"xk{i}") for i in range(ND)]
            xv_t = [wp.tile([P, S], BF16, name=f"xv{i}") for i in range(ND)]
            wqkv_t = [wp.tile([P, 3 * HDC], BF16, name=f"wqkv{i}") for i in range(ND)]
            wq_t = [wqkv_t[i][:, 0:HDC] for i in range(ND)]
            wk_t = [wqkv_t[i][:, HDC:2 * HDC] for i in range(ND)]
            wv_t = [wqkv_t[i][:, 2 * HDC:3 * HDC] for i in range(ND)]
            wo_t = [wp.tile([P, D], BF16, name=f"wo{i}") for i in range(NM)]
            mA = wp.tile([P, P], BF16, name="mA")
            onr = wp.tile([1, 2 * P], mybir.dt.float32r, name="onr")
            nc.sync.dma_start(onr[:], onesr[:, :])
            zb = wp.tile([P, 3 * P], BF16, name="zb")
            nc.vector.memset(zb[:], 0.0)
            qT = [wp.tile([P, S], BF16, name=f"qT{m}") for m in range(NM)]
            kT = [wp.tile([P, S], BF16, name=f"kT{m}") for m in range(NM)]
            oT = [wp.tile([P, S], BF16, name=f"oT{m}") for m in range(NM)]
            vaug = [wp.tile([P, 4 * P], BF16, name=f"vaug{t}") for t in range(NST)]

            # ---- DMAs: weights + vaug init, then x (block-0 quarter first) ----
            for i in range(ND):
                nc.sync.dma_start(wqkv_t[i][:], wqkv[i * P:(i + 1) * P, :])
            for i in range(NM):
                nc.sync.dma_start(wo_t[i][:], wod[i * P:(i + 1) * P, :])
            nc.sync.dma_start(mA[:], maskA[:, :])
            for t in range(NST):
                eng = nc.gpsimd if t % 2 == 0 else nc.scalar
                eng.dma_start(vaug[t][:], vinit[:, :])
            dq = [nc.sync, nc.gpsimd, nc.scalar]
            n = 0
            for xs, xd in ((xk_t, xkT), (xq_t, xqT), (xv_t, xvT)):
                for dd in range(ND):
                    dq[n % 3].dma_start(xs[dd][:, 0:CH], xd[dd * P:(dd + 1) * P, 0:CH])
                    n += 1
            for xs, xd in ((xk_t, xkT), (xq_t, xqT), (xv_t, xvT)):
                for dd in range(ND):
                    dq[n % 3].dma_start(xs[dd][:, CH:S], xd[dd * P:(dd + 1) * P, CH:S])
                    n += 1

            # ---- filler queue ----
            fq = deque()

            def pump(k):
                for _ in range(k):
                    if not fq:
                        return
                    fq.popleft()[1]()

            def drain(match):
                if not any(match(key) for key, _ in fq):
                    return
                while fq:
                    key, fn = fq.popleft()
                    fn()
                    if not any(match(k2) for k2, _ in fq):
                        return

            # ---- projection units ----
            def unit_kq(w_views, dstT, mi, xs, sb):
                def run():
                    pq = pp_ps.tile([P, CH], F32, name="pq", tag="pp")
                    for dd in range(ND):
                        nc.tensor.matmul(
                            pq[:], w_views[dd][:, mi * P:(mi + 1) * P],
                            xs[dd][:, sb * CH:(sb + 1) * CH],
                            start=(dd == 0), stop=(dd == ND - 1))
                    nc.vector.tensor_copy(dstT[mi][:, sb * CH:(sb + 1) * CH], pq[:])
                return run

            def unit_v(st):
                # natural-layout v for s-tile st: [128 seq, 256 dv] then scatter
                # into vaug[st]: A-dims -> [256mi, 256mi+64), B-dims -> [256mi+192, ...)
                def run():
                    pv = pp_ps.tile([P, HDC], F32, name="pv", tag="pp")
                    for dd in range(ND):
                        nc.tensor.matmul(
                            pv[:], xv_t[dd][:, st * P:(st + 1) * P], wv_t[dd][:],
                            start=(dd == 0), stop=(dd == ND - 1))
                    dstA = vaug[st].rearrange("p (mi x) -> p mi x", mi=2)[:, :, 0:DV]
                    srcA = pv[:, 0:P].rearrange("p (mi d) -> p mi d", mi=2)
                    nc.vector.tensor_copy(dstA, srcA)
                    dstB = vaug[st].rearrange("p (mi x) -> p mi x", mi=2)[:, :, 3 * DK:4 * DK]
                    srcB = pv[:, P:2 * P].rearrange("p (mi d) -> p mi d", mi=2)
                    nc.vector.tensor_copy(dstB, srcB)
                return run

            def enqueue_block(sb):
                for mi in range(NM):
                    fq.append((("q", sb), unit_kq(wq_t, qT, mi, xq_t, sb)))
                for mi in range(NM):
                    fq.append((("kv", sb), unit_kq(wk_t, kT, mi, xk_t, sb)))
                for st in range(4 * sb, 4 * sb + 4):
                    fq.append((("kv", sb), unit_v(st)))

            # ---- normalize + output projection units ----
            def unit_norm(c, mi, ovA, ovB):
                # head A: dims rows 0-63, den row 64; head B: dims 64-127, den 0.
                # rbT = ones(64x1) @ recip(den) rank-1 broadcasts per head into
                # one PSUM tile (A rows 0-63 at col-group 0, B at col-group 64,
                # concurrent); copied to SBUF once, then one fused
                # normalize-multiply per head out of ov PSUM.
                def run():
                    rT = rp.tile([P, CH], BF16, name="rT", tag="rT")
                    rbT = pp_ps.tile([P, CH], F32, name="rbT", tag="pp")
                    for hb, ov_tile, dr in ((0, ovA, DV), (1, ovB, 0)):
                        dstg = sp.tile([1, CH], F32, name="dstg", tag="dstg")
                        stg = sp.tile([1, CH], mybir.dt.float32r,
                                      name="stg", tag="stg")
                        nc.vector.tensor_copy(dstg[:], ov_tile[dr:dr + 1, :])
                        with nc.allow_low_precision(reason="softmax denom"):
                            nc.vector.reciprocal(stg[:], dstg[:])
                        # lhsT [1,128] = [ones|zeros] (A) / [zeros|ones] (B):
                        # rank-1 broadcast lands on that head's partition rows;
                        # the two accumulate into one PSUM tile
                        nc.tensor.matmul(
                            rbT[:], onr[:, hb * P:(hb + 1) * P], stg[:],
                            start=(hb == 0), stop=(hb == 1))
                    nc.scalar.copy(rT[:], rbT[:])
                    for hb, ov_tile in ((0, ovA), (1, ovB)):
                        rows = slice(0, DV) if hb == 0 else slice(DV, P)
                        nc.vector.tensor_mul(
                            oT[mi][rows, c * CH:(c + 1) * CH],
                            ov_tile[rows, :], rT[rows, :])
                return run

            def unit_oproj(c, st):
                def run():
                    ob = obp.tile([P, D], BF16, name="ob", tag="ob")
                    for nh in range(2):
                        pq = pp_ps.tile([P, CH], F32, name="po", tag="pp")
                        for mi in range(NM):
                            nc.tensor.matmul(
                                pq[:], oT[mi][:, st * P:(st + 1) * P],
                                wo_t[mi][:, nh * CH:(nh + 1) * CH],
                                start=(mi == 0), stop=(mi == NM - 1))
                        nc.vector.tensor_copy(ob[:, nh * CH:(nh + 1) * CH], pq[:])
                    eng = nc.sync if st % 2 == 0 else nc.gpsimd
                    eng.dma_start(out[st * P:(st + 1) * P, :], ob[:])
                return run

            # ---- attention ----
            def attention_chain(c, mi):
                nt = 4 * c + 4
                ovA = ov_ps.tile([P, CH], F32, name="ovA", tag="ov")
                ovB = ov_ps.tile([P, CH], F32, name="ovB", tag="ov")
                reg = reg_ps.tile([P, 4 * CH], F32, name="reg", tag="reg")
                for s in range(nt // 2):
                    t0 = 2 * s
                    drain(lambda k, b=t0 // 4: k[0] == "kv" and k[1] <= b)
                    # scores: A/B pairs back-to-back -> concurrent row groups
                    for j in range(2):
                        t = t0 + j
                        for hb in range(2):
                            co = hb * 2 * CH + j * CH
                            nc.tensor.matmul(
                                reg[:, co:co + CH],
                                kT[mi][hb * DK:(hb + 1) * DK, t * P:(t + 1) * P],
                                qT[mi][hb * DK:(hb + 1) * DK, c * CH:(c + 1) * CH],
                                start=True, stop=True)
                    pump(1)
                    ex = ep.tile([P, 4 * CH], BF16, name="ex", tag="ex")
                    for eh in range(4):  # per-PSUM-bank (bank-crossing reads TBD)
                        nc.scalar.activation(ex[:, eh * CH:(eh + 1) * CH],
                                             reg[:, eh * CH:(eh + 1) * CH], Exp)
                    # diagonal-block masks (DVE; gpsimd semaphores cost ~7us)
                    for j in range(2):
                        r = t0 + j - 4 * c
                        if r >= 0:
                            for hb in range(2):
                                co = hb * 2 * CH + j * CH + r * P
                                nc.vector.tensor_mul(
                                    ex[:, co:co + P], ex[:, co:co + P], mA[:])
                    # attn@v (+denominator); masked column prefixes zero-filled
                    # (partial-column PSUM accumulation mis-executes on HW)
                    for j in range(2):
                        t = t0 + j
                        r = t - 4 * c
                        lo = max(r, 0) * P
                        if lo > 0:
                            for hb in range(2):
                                co = hb * 2 * CH + j * CH
                                nc.vector.tensor_copy(ex[:, co:co + lo], zb[:, 0:lo])
                        for hb, ov in ((0, ovA), (1, ovB)):
                            co = hb * 2 * CH + j * CH
                            nc.tensor.matmul(
                                ov[:],
                                vaug[t][:, mi * 2 * P + hb * P:mi * 2 * P + (hb + 1) * P],
                                ex[:, co:co + CH],
                                start=(t == 0), stop=(t == nt - 1))
                    pump(1)
                # emitted directly: the next chain's attn@v recycles these ov
                # slots, so their normalize must precede it in every stream
                unit_norm(c, mi, ovA, ovB)()

            # ---- main pipeline ----
            enqueue_block(0)
            drain(lambda k: k[0] in ("q", "kv") and k[1] == 0)
            for c in range(NCH):
                if c + 1 < NCH:
                    enqueue_block(c + 1)
                drain(lambda k, c=c: k[0] == "q" and k[1] <= c)
                for mi in range(NM):
                    attention_chain(c, mi)
                for st in range(4 * c, 4 * c + 4):
                    fq.append((("no", c), unit_oproj(c, st)))
            while fq:
                fq.popleft()[1]()
    nc.compile()
    return nc


_NC_CACHE = {}
LAST_RESULT = None


def _get_nc():
    if "nc" not in _NC_CACHE:
        import concourse.tile as tile
        import concourse.mybir as mybir
        from concourse import bacc
        nc = bacc.Bacc("TRN2", target_bir_lowering=False, num_devices=NCORES)
        _NC_CACHE["nc"] = build(nc, tile, mybir)
    return _NC_CACHE["nc"]


def onesr_pat():
    o = np.zeros((1, 2 * P), dtype=np.float32)
    o[0, 0:DV] = 1.0           # A: rows 0-63
    o[0, P + DV:2 * P] = 1.0   # B: rows 64-127
    return o


def make_consts():
    import ml_dtypes
    i = np.arange(P)[:, None]
    j = np.arange(P)[None, :]
    maskA = (j >= i).astype(ml_dtypes.bfloat16)
    vinit = np.zeros((P, 4 * P), dtype=np.float32)
    for mi in range(NM):
        vinit[:, mi * 2 * P + DV] = 1.0  # A ones col -> den row 64
        vinit[:, mi * 2 * P + P] = 1.0   # B ones col -> den row 0
    return maskA, vinit.astype(ml_dtypes.bfloat16)


def kernel(Q, K, V, Wq, Wk, Wv, Wo):
    import ml_dtypes
    from concourse.bass_utils import run_bass_kernel_spmd

    BF = ml_dtypes.bfloat16
    Q = np.asarray(Q, dtype=np.float32)
    K = np.asarray(K, dtype=np.float32)
    V = np.asarray(V, dtype=np.float32)
    Wq = np.asarray(Wq, dtype=np.float32) * np.float32(1.0 / np.sqrt(DK))
    Wk = np.asarray(Wk, dtype=np.float32)
    Wv = np.asarray(Wv, dtype=np.float32)
    Wo = np.asarray(Wo, dtype=np.float32)

    QT = [np.ascontiguousarray(Q[b].T).astype(BF) for b in range(B)]
    KT = [np.ascontiguousarray(K[b].T).astype(BF) for b in range(B)]
    VT = [np.ascontiguousarray(V[b].T).astype(BF) for b in range(B)]
    maskA, vinit = make_consts()

    in_maps = []
    for core in range(NCORES):
        b, g = core // HG, core % HG
        cs = slice(g * HDC, (g + 1) * HDC)
        wv_c = Wv[:, cs].reshape(D, HPC, DK)
        # head order [h0, h2, h1, h3]: A-dims then B-dims per pair
        wv_c = wv_c[:, [0, 2, 1, 3], :].reshape(D, HDC)
        in_maps.append({
            "xqT": QT[b], "xkT": KT[b], "xvT": VT[b],
            "wqkv": np.ascontiguousarray(
                np.concatenate([Wq[:, cs], Wk[:, cs], wv_c], axis=1)).astype(BF),
            "wod": np.ascontiguousarray(Wo[cs, :]).astype(BF),
            "maskA": maskA, "vinit": vinit,
            "onesr": onesr_pat(),
        })

    nc = _get_nc()
    res = run_bass_kernel_spmd(nc, in_maps, core_ids=list(range(NCORES)))
    global LAST_RESULT
    LAST_RESULT = res

    acc = np.zeros((B, S, D), dtype=np.float64)
    for core in range(NCORES):
        acc[core // HG] += res.results[core]["out"].astype(np.float64)
    return acc.astype(np.float32)


# revision 27
# speedup vs baseline: 1.2506x; 1.2506x over previous
"""Multi-head causal attention (B=2, S=2048, D=1024, H=16, DK=DV=64) on 8 Trainium2
NeuronCores.

Sharding: 2-way batch x 4-way head-group. Core i handles batch i//4 and heads
[4*(i%4), 4*(i%4)+4). Each core projects q/k/v for its head group, runs causal
attention, and computes a partial output projection through its row-block of Wo.
The 4 partial outputs per batch are summed on the host.

v2 design (vs the f32r baseline):
- Everything bf16 (inputs downcast on host): full-rate PE, FWL weight loads,
  half the HBM traffic. PSUM accumulation stays fp32.
- Scores for a head PAIR run concurrently on the PE via row-group tiling
  (K=64 each; heads 2mi/2mi+1 live on partitions 0-63/64-127 of qT/kT).
- v is projected in natural [seq, dv] layout (no PE transposes) and scattered
  into padded per-k-tile lhsT blocks: A = [dims(64)|ones|zeros(63)],
  B = [ones|zeros(63)|dims(64)]. attn@v with M=128 then lands head A's
  numerator on PSUM partitions 0-63 (denominator row 64) and head B's on
  64-127 (denominator row 0), so the softmax normalize multiply is a single
  partition-aligned DVE op per head fused with the PSUM->SBUF cast.
- Scores for 2 k-tiles x 2 heads accumulate into one [128, 2048] PSUM region
  (4 banks); exp runs as two [128,1024] ACT instructions (A-half / B-half) so
  the next step's A-scores only wait on the A-half exp (subtile deps).
- Denominator reciprocal via the fast approx DVE op straight out of PSUM;
  broadcast across 64 partitions with a gpsimd partition_broadcast.
- Causal masking: diagonal 128-blocks get a triangular mask multiply on
  gpsimd (SBUF bf16); fully-masked column prefixes are simply skipped by
  restricting the attn@v matmul to [lo:512].
- The whole kernel is software-pipelined with an explicit filler queue:
  projection / normalize / output-projection work is emitted between the
  attention steps so the PE never idles behind the ACT exp chain.
"""
import sys

sys.path.insert(0, "/opt/trn_rl_repo")
import numpy as np

B, S, D = 2, 2048, 1024
H, DK, DV = 16, 64, 64
NCORES = 8
HG = 4          # head-group cores per batch
HPC = 4         # heads per core
HDC = HPC * DK  # 256 projection cols per core
P = 128         # partitions
CH = 512        # q-chunk size
NCH = S // CH   # 4 q-chunks (also 512-col seq blocks)
NST = S // P    # 16 s-tiles (k-tiles)
ND = D // P     # 8 d-tiles
NM = 2          # head pairs


def build(nc, tile, mybir):
    from collections import deque
    from contextlib import ExitStack

    BF16 = mybir.dt.bfloat16
    F32 = mybir.dt.float32
    Exp = mybir.ActivationFunctionType.Exp

    xqT = nc.dram_tensor("xqT", [D, S], BF16, kind="ExternalInput").ap()
    xkT = nc.dram_tensor("xkT", [D, S], BF16, kind="ExternalInput").ap()
    xvT = nc.dram_tensor("xvT", [D, S], BF16, kind="ExternalInput").ap()
    wqkv = nc.dram_tensor("wqkv", [D, 3 * HDC], BF16, kind="ExternalInput").ap()
    wod = nc.dram_tensor("wod", [HDC, D], BF16, kind="ExternalInput").ap()
    maskA = nc.dram_tensor("maskA", [P, P], BF16, kind="ExternalInput").ap()
    vinit = nc.dram_tensor("vinit", [P, 4 * P], BF16, kind="ExternalInput").ap()
    onesr = nc.dram_tensor("onesr", [1, 2 * P], mybir.dt.float32r,
                           kind="ExternalInput").ap()
    out = nc.dram_tensor("out", [S, D], BF16, kind="ExternalOutput").ap()

    with tile.TileContext(nc) as tc:
        with ExitStack() as ctx:
            wp = ctx.enter_context(tc.tile_pool(name="wp", bufs=1))
            ep = ctx.enter_context(tc.tile_pool(name="ep", bufs=2))
            obp = ctx.enter_context(tc.tile_pool(name="obp", bufs=2))
            sp = ctx.enter_context(tc.tile_pool(name="sp", bufs=4))
            rp = ctx.enter_context(tc.tile_pool(name="rp", bufs=2))
            pp_ps = ctx.enter_context(tc.tile_pool(name="pp_ps", bufs=2, space="PSUM"))
            reg_ps = ctx.enter_context(tc.tile_pool(name="reg_ps", bufs=1, space="PSUM"))
            ov_ps = ctx.enter_context(tc.tile_pool(name="ov_ps", bufs=2, space="PSUM"))

            # ---- persistent SBUF tiles ----
            xq_t = [wp.tile([P, S], BF16, name=f"xq{i}") for i in range(ND)]
            xk_t = [wp.tile([P, S], BF16, name=f"xk{i}") for i in range(ND)]
            xv_t = [wp.tile([P, S], BF16, name=f"xv{i}") for i in range(ND)]
            wqkv_t = [wp.tile([P, 3 * HDC], BF16, name=f"wqkv{i}") for i in range(ND)]
            wq_t = [wqkv_t[i][:, 0:HDC] for i in range(ND)]
            wk_t = [wqkv_t[i][:, HDC:2 * HDC] for i in range(ND)]
            wv_t = [wqkv_t[i][:, 2 * HDC:3 * HDC] for i in range(ND)]
            wo_t = [wp.tile([P, D], BF16, name=f"wo{i}") for i in range(NM)]
            mA = wp.tile([P, P], BF16, name="mA")
            onr = wp.tile([1, 2 * P], mybir.dt.float32r, name="onr")
            nc.sync.dma_start(onr[:], onesr[:, :])
            zb = wp.tile([P, 3 * P], BF16, name="zb")
            nc.vector.memset(zb[:], 0.0)
            qT = [wp.tile([P, S], BF16, name=f"qT{m}") for m in range(NM)]
            kT = [wp.tile([P, S], BF16, name=f"kT{m}") for m in range(NM)]
            oT = [wp.tile([P, S], BF16, name=f"oT{m}") for m in range(NM)]
            vaug = [wp.tile([P, 4 * P], BF16, name=f"vaug{t}") for t in range(NST)]

            # ---- DMAs: weights + vaug init, then x (block-0 quarter first) ----
            for i in range(ND):
                nc.sync.dma_start(wqkv_t[i][:], wqkv[i * P:(i + 1) * P, :])
            for i in range(NM):
                nc.sync.dma_start(wo_t[i][:], wod[i * P:(i + 1) * P, :])
            nc.sync.dma_start(mA[:], maskA[:, :])
            for t in range(NST):
                eng = nc.gpsimd if t % 2 == 0 else nc.scalar
                eng.dma_start(vaug[t][:], vinit[:, :])
            dq = [nc.sync, nc.gpsimd, nc.scalar]
            n = 0
            for xs, xd in ((xk_t, xkT), (xq_t, xqT), (xv_t, xvT)):
                for dd in range(ND):
                    dq[n % 3].dma_start(xs[dd][:, 0:CH], xd[dd * P:(dd + 1) * P, 0:CH])
                    n += 1
            for xs, xd in ((xk_t, xkT), (xq_t, xqT), (xv_t, xvT)):
                for dd in range(ND):
                    dq[n % 3].dma_start(xs[dd][:, CH:S], xd[dd * P:(dd + 1) * P, CH:S])
                    n += 1

            # ---- filler queue ----
            fq = deque()

            def pump(k):
                for _ in range(k):
                    if not fq:
                        return
                    fq.popleft()[1]()

            def drain(match):
                if not any(match(key) for key, _ in fq):
                    return
                while fq:
                    key, fn = fq.popleft()
                    fn()
                    if not any(match(k2) for k2, _ in fq):
                        return

            # ---- projection units ----
            def unit_kq(w_views, dstT, mi, xs, sb):
                def run():
                    pq = pp_ps.tile([P, CH], F32, name="pq", tag="pp")
                    for dd in range(ND):
                        nc.tensor.matmul(
                            pq[:], w_views[dd][:, mi * P:(mi + 1) * P],
                            xs[dd][:, sb * CH:(sb + 1) * CH],
                            start=(dd == 0), stop=(dd == ND - 1))
                    nc.vector.tensor_copy(dstT[mi][:, sb * CH:(sb + 1) * CH], pq[:])
                return run

            def unit_v(st):
                # natural-layout v for s-tile st: [128 seq, 256 dv] then scatter
                # into vaug[st]: A-dims -> [256mi, 256mi+64), B-dims -> [256mi+192, ...)
                def run():
                    pv = pp_ps.tile([P, HDC], F32, name="pv", tag="pp")
                    for dd in range(ND):
                        nc.tensor.matmul(
                            pv[:], xv_t[dd][:, st * P:(st + 1) * P], wv_t[dd][:],
                            start=(dd == 0), stop=(dd == ND - 1))
                    dstA = vaug[st].rearrange("p (mi x) -> p mi x", mi=2)[:, :, 0:DV]
                    srcA = pv[:, 0:P].rearrange("p (mi d) -> p mi d", mi=2)
                    nc.vector.tensor_copy(dstA, srcA)
                    dstB = vaug[st].rearrange("p (mi x) -> p mi x", mi=2)[:, :, 3 * DK:4 * DK]
                    srcB = pv[:, P:2 * P].rearrange("p (mi d) -> p mi d", mi=2)
                    nc.vector.tensor_copy(dstB, srcB)
                return run

            def enqueue_block(sb):
                for mi in range(NM):
                    fq.append((("q", sb), unit_kq(wq_t, qT, mi, xq_t, sb)))
                for mi in range(NM):
                    fq.append((("kv", sb), unit_kq(wk_t, kT, mi, xk_t, sb)))
                for st in range(4 * sb, 4 * sb + 4):
                    fq.append((("kv", sb), unit_v(st)))

            # ---- normalize + output projection units ----
            def unit_norm(c, mi, ovA, ovB):
                # head A: dims rows 0-63, den row 64; head B: dims 64-127, den 0.
                # rbT = ones(64x1) @ recip(den) rank-1 broadcasts per head into
                # one PSUM tile (A rows 0-63 at col-group 0, B at col-group 64,
                # concurrent); copied to SBUF once, then one fused
                # normalize-multiply per head out of ov PSUM.
                def run():
                    rT = rp.tile([P, CH], BF16, name="rT", tag="rT")
                    rbT = pp_ps.tile([P, CH], F32, name="rbT", tag="pp")
                    for hb, ov_tile, dr in ((0, ovA, DV), (1, ovB, 0)):
                        dstg = sp.tile([1, CH], F32, name="dstg", tag="dstg")
                        stg = sp.tile([1, CH], mybir.dt.float32r,
                                      name="stg", tag="stg")
                        nc.vector.tensor_copy(dstg[:], ov_tile[dr:dr + 1, :])
                        # reciprocal_approx_fast with f32r out (same bit
                        # layout; the helper's f32-only assert is over-strict)
                        from concourse.dve_ops import (
                            RECIP_APPROX_FAST_CONSTS, RECIPROCAL_APPROX_FAST)
                        cst = RECIP_APPROX_FAST_CONSTS
                        nc.vector._custom_dve(
                            RECIPROCAL_APPROX_FAST, out=stg[:], in0=dstg[:],
                            s0=cst["s0"], s1=cst["s1"], imm2=cst["imm2"])
                        # lhsT [1,128] = [ones|zeros] (A) / [zeros|ones] (B):
                        # rank-1 broadcast lands on that head's partition rows;
                        # the two accumulate into one PSUM tile
                        nc.tensor.matmul(
                            rbT[:], onr[:, hb * P:(hb + 1) * P], stg[:],
                            start=(hb == 0), stop=(hb == 1))
                    nc.scalar.copy(rT[:], rbT[:])
                    for hb, ov_tile in ((0, ovA), (1, ovB)):
                        rows = slice(0, DV) if hb == 0 else slice(DV, P)
                        nc.vector.tensor_mul(
                            oT[mi][rows, c * CH:(c + 1) * CH],
                            ov_tile[rows, :], rT[rows, :])
                return run

            def unit_oproj(c, st):
                def run():
                    ob = obp.tile([P, D], BF16, name="ob", tag="ob")
                    for nh in range(2):
                        pq = pp_ps.tile([P, CH], F32, name="po", tag="pp")
                        for mi in range(NM):
                            nc.tensor.matmul(
                                pq[:], oT[mi][:, st * P:(st + 1) * P],
                                wo_t[mi][:, nh * CH:(nh + 1) * CH],
                                start=(mi == 0), stop=(mi == NM - 1))
                        nc.vector.tensor_copy(ob[:, nh * CH:(nh + 1) * CH], pq[:])
                    eng = nc.sync if st % 2 == 0 else nc.gpsimd
                    eng.dma_start(out[st * P:(st + 1) * P, :], ob[:])
                return run

            # ---- attention ----
            def attention_chain(c, mi):
                nt = 4 * c + 4
                ovA = ov_ps.tile([P, CH], F32, name="ovA", tag="ov")
                ovB = ov_ps.tile([P, CH], F32, name="ovB", tag="ov")
                reg = reg_ps.tile([P, 4 * CH], F32, name="reg", tag="reg")
                for s in range(nt // 2):
                    t0 = 2 * s
                    drain(lambda k, b=t0 // 4: k[0] == "kv" and k[1] <= b)
                    # scores: A/B pairs back-to-back -> concurrent row groups
                    for j in range(2):
                        t = t0 + j
                        for hb in range(2):
                            co = hb * 2 * CH + j * CH
                            nc.tensor.matmul(
                                reg[:, co:co + CH],
                                kT[mi][hb * DK:(hb + 1) * DK, t * P:(t + 1) * P],
                                qT[mi][hb * DK:(hb + 1) * DK, c * CH:(c + 1) * CH],
                                start=True, stop=True)
                    pump(1)
                    ex = ep.tile([P, 4 * CH], BF16, name="ex", tag="ex")
                    for eh in range(2):  # half-region exps (A-half / B-half)
                        nc.scalar.activation(ex[:, eh * 2 * CH:(eh + 1) * 2 * CH],
                                             reg[:, eh * 2 * CH:(eh + 1) * 2 * CH],
                                             Exp)
                    # diagonal-block masks (DVE; gpsimd semaphores cost ~7us)
                    for j in range(2):
                        r = t0 + j - 4 * c
                        if r >= 0:
                            for hb in range(2):
                                co = hb * 2 * CH + j * CH + r * P
                                nc.vector.tensor_mul(
                                    ex[:, co:co + P], ex[:, co:co + P], mA[:])
                    # attn@v (+denominator); masked column prefixes zero-filled
                    # (partial-column PSUM accumulation mis-executes on HW)
                    for j in range(2):
                        t = t0 + j
                        r = t - 4 * c
                        lo = max(r, 0) * P
                        if lo > 0:
                            for hb in range(2):
                                co = hb * 2 * CH + j * CH
                                nc.vector.tensor_copy(ex[:, co:co + lo], zb[:, 0:lo])
                        for hb, ov in ((0, ovA), (1, ovB)):
                            co = hb * 2 * CH + j * CH
                            nc.tensor.matmul(
                                ov[:],
                                vaug[t][:, mi * 2 * P + hb * P:mi * 2 * P + (hb + 1) * P],
                                ex[:, co:co + CH],
                                start=(t == 0), stop=(t == nt - 1))
                    pump(1)
                # emitted directly: the next chain's attn@v recycles these ov
                # slots, so their normalize must precede it in every stream
                unit_norm(c, mi, ovA, ovB)()

            # ---- main pipeline ----
            enqueue_block(0)
            drain(lambda k: k[0] in ("q", "kv") and k[1] == 0)
            for c in range(NCH):
                if c + 1 < NCH:
                    enqueue_block(c + 1)
                drain(lambda k, c=c: k[0] == "q" and k[1] <= c)
                for mi in range(NM):
                    attention_chain(c, mi)
                for st in range(4 * c, 4 * c + 4):
                    fq.append((("no", c), unit_oproj(c, st)))
            while fq:
                fq.popleft()[1]()
    nc.compile()
    return nc


_NC_CACHE = {}
LAST_RESULT = None


def _get_nc():
    if "nc" not in _NC_CACHE:
        import concourse.tile as tile
        import concourse.mybir as mybir
        from concourse import bacc
        nc = bacc.Bacc("TRN2", target_bir_lowering=False, num_devices=NCORES)
        _NC_CACHE["nc"] = build(nc, tile, mybir)
    return _NC_CACHE["nc"]


def onesr_pat():
    o = np.zeros((1, 2 * P), dtype=np.float32)
    o[0, 0:DV] = 1.0           # A: rows 0-63
    o[0, P + DV:2 * P] = 1.0   # B: rows 64-127
    return o


def make_consts():
    import ml_dtypes
    i = np.arange(P)[:, None]
    j = np.arange(P)[None, :]
    maskA = (j >= i).astype(ml_dtypes.bfloat16)
    vinit = np.zeros((P, 4 * P), dtype=np.float32)
    for mi in range(NM):
        vinit[:, mi * 2 * P + DV] = 1.0  # A ones col -> den row 64
        vinit[:, mi * 2 * P + P] = 1.0   # B ones col -> den row 0
    return maskA, vinit.astype(ml_dtypes.bfloat16)


def kernel(Q, K, V, Wq, Wk, Wv, Wo):
    import ml_dtypes
    from concourse.bass_utils import run_bass_kernel_spmd

    BF = ml_dtypes.bfloat16
    Q = np.asarray(Q, dtype=np.float32)
    K = np.asarray(K, dtype=np.float32)
    V = np.asarray(V, dtype=np.float32)
    Wq = np.asarray(Wq, dtype=np.float32) * np.float32(1.0 / np.sqrt(DK))
    Wk = np.asarray(Wk, dtype=np.float32)
    Wv = np.asarray(Wv, dtype=np.float32)
    Wo = np.asarray(Wo, dtype=np.float32)

    QT = [np.ascontiguousarray(Q[b].T).astype(BF) for b in range(B)]
    KT = [np.ascontiguousarray(K[b].T).astype(BF) for b in range(B)]
    VT = [np.ascontiguousarray(V[b].T).astype(BF) for b in range(B)]
    maskA, vinit = make_consts()

    in_maps = []
    for core in range(NCORES):
        b, g = core // HG, core % HG
        cs = slice(g * HDC, (g + 1) * HDC)
        wv_c = Wv[:, cs].reshape(D, HPC, DK)
        # head order [h0, h2, h1, h3]: A-dims then B-dims per pair
        wv_c = wv_c[:, [0, 2, 1, 3], :].reshape(D, HDC)
        in_maps.append({
            "xqT": QT[b], "xkT": KT[b], "xvT": VT[b],
            "wqkv": np.ascontiguousarray(
                np.concatenate([Wq[:, cs], Wk[:, cs], wv_c], axis=1)).astype(BF),
            "wod": np.ascontiguousarray(Wo[cs, :]).astype(BF),
            "maskA": maskA, "vinit": vinit,
            "onesr": onesr_pat(),
        })

    nc = _get_nc()
    res = run_bass_kernel_spmd(nc, in_maps, core_ids=list(range(NCORES)))
    global LAST_RESULT
    LAST_RESULT = res

    acc = np.zeros((B, S, D), dtype=np.float64)
    for core in range(NCORES):
        acc[core // HG] += res.results[core]["out"].astype(np.float64)
    return acc.astype(np.float32)


# revision 31
# speedup vs baseline: 1.3824x; 1.1054x over previous
"""Multi-head causal attention (B=2, S=2048, D=1024, H=16, DK=DV=64) on 8 Trainium2
NeuronCores.

Sharding: 2-way batch x 4-way head-group. Core i handles batch i//4 and heads
[4*(i%4), 4*(i%4)+4). Each core projects q/k/v for its head group, runs causal
attention, and computes a partial output projection through its row-block of Wo.
The 4 partial outputs per batch are summed on the host.

v2 design (vs the f32r baseline):
- Everything bf16 (inputs downcast on host): full-rate PE, FWL weight loads,
  half the HBM traffic. PSUM accumulation stays fp32.
- Scores for a head PAIR run concurrently on the PE via row-group tiling
  (K=64 each; heads 2mi/2mi+1 live on partitions 0-63/64-127 of qT/kT).
- v is projected in natural [seq, dv] layout (no PE transposes) and scattered
  into padded per-k-tile lhsT blocks: A = [dims(64)|ones|zeros(63)],
  B = [ones|zeros(63)|dims(64)]. attn@v with M=128 then lands head A's
  numerator on PSUM partitions 0-63 (denominator row 64) and head B's on
  64-127 (denominator row 0), so the softmax normalize multiply is a single
  partition-aligned DVE op per head fused with the PSUM->SBUF cast.
- Scores for 2 k-tiles x 2 heads accumulate into one [128, 2048] PSUM region
  (4 banks); exp runs as two [128,1024] ACT instructions (A-half / B-half) so
  the next step's A-scores only wait on the A-half exp (subtile deps).
- Denominator reciprocal via the fast approx DVE op straight out of PSUM;
  broadcast across 64 partitions with a gpsimd partition_broadcast.
- Causal masking: diagonal 128-blocks get a triangular mask multiply on
  gpsimd (SBUF bf16); fully-masked column prefixes are simply skipped by
  restricting the attn@v matmul to [lo:512].
- The whole kernel is software-pipelined with an explicit filler queue:
  projection / normalize / output-projection work is emitted between the
  attention steps so the PE never idles behind the ACT exp chain.
"""
import sys

sys.path.insert(0, "/opt/trn_rl_repo")
import numpy as np

B, S, D = 2, 2048, 1024
H, DK, DV = 16, 64, 64
NCORES = 8
HG = 4          # head-group cores per batch
HPC = 4         # heads per core
HDC = HPC * DK  # 256 projection cols per core
P = 128         # partitions
CH = 512        # q-chunk size
NCH = S // CH   # 4 q-chunks (also 512-col seq blocks)
NST = S // P    # 16 s-tiles (k-tiles)
ND = D // P     # 8 d-tiles
NM = 2          # head pairs


def build(nc, tile, mybir):
    from collections import deque
    from contextlib import ExitStack

    BF16 = mybir.dt.bfloat16
    F32 = mybir.dt.float32
    Exp = mybir.ActivationFunctionType.Exp

    xqT = nc.dram_tensor("xqT", [D, S], BF16, kind="ExternalInput").ap()
    xkT = nc.dram_tensor("xkT", [D, S], BF16, kind="ExternalInput").ap()
    xvT = nc.dram_tensor("xvT", [D, S], BF16, kind="ExternalInput").ap()
    wqkv = nc.dram_tensor("wqkv", [D, 3 * HDC], BF16, kind="ExternalInput").ap()
    wod = nc.dram_tensor("wod", [HDC, D], BF16, kind="ExternalInput").ap()
    maskA = nc.dram_tensor("maskA", [P, P], BF16, kind="ExternalInput").ap()
    vinit = nc.dram_tensor("vinit", [P, 4 * P], BF16, kind="ExternalInput").ap()
    onesr = nc.dram_tensor("onesr", [1, 2 * P], mybir.dt.float32r,
                           kind="ExternalInput").ap()
    out = nc.dram_tensor("out", [S, D], BF16, kind="ExternalOutput").ap()

    with tile.TileContext(nc) as tc:
        with ExitStack() as ctx:
            wp = ctx.enter_context(tc.tile_pool(name="wp", bufs=1))
            ep = ctx.enter_context(tc.tile_pool(name="ep", bufs=3))
            obp = ctx.enter_context(tc.tile_pool(name="obp", bufs=2))
            sp = ctx.enter_context(tc.tile_pool(name="sp", bufs=4))
            rp = ctx.enter_context(tc.tile_pool(name="rp", bufs=2))
            pp_ps = ctx.enter_context(tc.tile_pool(name="pp_ps", bufs=2, space="PSUM"))
            reg_ps = ctx.enter_context(tc.tile_pool(name="reg_ps", bufs=2, space="PSUM"))
            ov_ps = ctx.enter_context(tc.tile_pool(name="ov_ps", bufs=2, space="PSUM"))

            # ---- persistent SBUF tiles ----
            xq_t = [wp.tile([P, S], BF16, name=f"xq{i}") for i in range(ND)]
            xk_t = [wp.tile([P, S], BF16, name=f"xk{i}") for i in range(ND)]
            xv_t = [wp.tile([P, S], BF16, name=f"xv{i}") for i in range(ND)]
            wqkv_t = [wp.tile([P, 3 * HDC], BF16, name=f"wqkv{i}") for i in range(ND)]
            wq_t = [wqkv_t[i][:, 0:HDC] for i in range(ND)]
            wk_t = [wqkv_t[i][:, HDC:2 * HDC] for i in range(ND)]
            wv_t = [wqkv_t[i][:, 2 * HDC:3 * HDC] for i in range(ND)]
            wo_t = [wp.tile([P, D], BF16, name=f"wo{i}") for i in range(NM)]
            mA = wp.tile([P, P], BF16, name="mA")
            onr = wp.tile([1, 2 * P], mybir.dt.float32r, name="onr")
            nc.sync.dma_start(onr[:], onesr[:, :])
            zb = wp.tile([P, 3 * P], BF16, name="zb")
            nc.vector.memset(zb[:], 0.0)
            qT = [wp.tile([P, S], BF16, name=f"qT{m}") for m in range(NM)]
            kT = [wp.tile([P, S], BF16, name=f"kT{m}") for m in range(NM)]
            oT = [wp.tile([P, S], BF16, name=f"oT{m}") for m in range(NM)]
            vaug = [wp.tile([P, 4 * P], BF16, name=f"vaug{t}") for t in range(NST)]

            # ---- DMAs: weights + vaug init, then x (block-0 quarter first) ----
            for i in range(ND):
                nc.sync.dma_start(wqkv_t[i][:], wqkv[i * P:(i + 1) * P, :])
            for i in range(NM):
                nc.sync.dma_start(wo_t[i][:], wod[i * P:(i + 1) * P, :])
            nc.sync.dma_start(mA[:], maskA[:, :])
            for t in range(NST):
                eng = nc.gpsimd if t % 2 == 0 else nc.scalar
                eng.dma_start(vaug[t][:], vinit[:, :])
            dq = [nc.sync, nc.gpsimd]
            n = 0
            for cols in (slice(0, 2 * CH), slice(2 * CH, S)):
                for xs, xd in ((xk_t, xkT), (xq_t, xqT), (xv_t, xvT)):
                    for dd in range(ND):
                        dq[n % 2].dma_start(xs[dd][:, cols],
                                            xd[dd * P:(dd + 1) * P, cols])
                        n += 1

            # ---- filler queue ----
            fq = deque()

            def pump(k):
                for _ in range(k):
                    if not fq:
                        return
                    fq.popleft()[1]()

            def drain(match):
                if not any(match(key) for key, _ in fq):
                    return
                while fq:
                    key, fn = fq.popleft()
                    fn()
                    if not any(match(k2) for k2, _ in fq):
                        return

            # ---- projection units ----
            def unit_kq(w_views, dstT, mi, xs, sb):
                def run():
                    pq = pp_ps.tile([P, CH], F32, name="pq", tag="pp")
                    for dd in range(ND):
                        nc.tensor.matmul(
                            pq[:], w_views[dd][:, mi * P:(mi + 1) * P],
                            xs[dd][:, sb * CH:(sb + 1) * CH],
                            start=(dd == 0), stop=(dd == ND - 1))
                    nc.vector.tensor_copy(dstT[mi][:, sb * CH:(sb + 1) * CH], pq[:])
                return run

            def unit_v(st):
                # natural-layout v for s-tile st: [128 seq, 256 dv] then scatter
                # into vaug[st]: A-dims -> [256mi, 256mi+64), B-dims -> [256mi+192, ...)
                def run():
                    pv = pp_ps.tile([P, HDC], F32, name="pv", tag="pp")
                    for dd in range(ND):
                        nc.tensor.matmul(
                            pv[:], xv_t[dd][:, st * P:(st + 1) * P], wv_t[dd][:],
                            start=(dd == 0), stop=(dd == ND - 1))
                    dstA = vaug[st].rearrange("p (mi x) -> p mi x", mi=2)[:, :, 0:DV]
                    srcA = pv[:, 0:P].rearrange("p (mi d) -> p mi d", mi=2)
                    nc.vector.tensor_copy(dstA, srcA)
                    dstB = vaug[st].rearrange("p (mi x) -> p mi x", mi=2)[:, :, 3 * DK:4 * DK]
                    srcB = pv[:, P:2 * P].rearrange("p (mi d) -> p mi d", mi=2)
                    nc.vector.tensor_copy(dstB, srcB)
                return run

            def enqueue_block(sb):
                for mi in range(NM):
                    fq.append((("q", sb), unit_kq(wq_t, qT, mi, xq_t, sb)))
                for mi in range(NM):
                    fq.append((("kv", sb), unit_kq(wk_t, kT, mi, xk_t, sb)))
                for st in range(4 * sb, 4 * sb + 4):
                    fq.append((("kv", sb), unit_v(st)))

            # ---- normalize + output projection units ----
            def unit_norm(c, mi, ovA, ovB):
                # head A: dims rows 0-63, den row 64; head B: dims 64-127, den 0.
                # rbT = ones(64x1) @ recip(den) rank-1 broadcasts per head into
                # one PSUM tile (A rows 0-63 at col-group 0, B at col-group 64,
                # concurrent); copied to SBUF once, then one fused
                # normalize-multiply per head out of ov PSUM.
                def run():
                    rT = rp.tile([P, CH], BF16, name="rT", tag="rT")
                    rbT = pp_ps.tile([P, CH], F32, name="rbT", tag="pp")
                    for hb, ov_tile, dr in ((0, ovA, DV), (1, ovB, 0)):
                        dstg = sp.tile([1, CH], F32, name="dstg", tag="dstg")
                        stg = sp.tile([1, CH], mybir.dt.float32r,
                                      name="stg", tag="stg")
                        nc.vector.tensor_copy(dstg[:], ov_tile[dr:dr + 1, :])
                        # reciprocal_approx_fast with f32r out (same bit
                        # layout; the helper's f32-only assert is over-strict)
                        from concourse.dve_ops import (
                            RECIP_APPROX_FAST_CONSTS, RECIPROCAL_APPROX_FAST)
                        cst = RECIP_APPROX_FAST_CONSTS
                        nc.vector._custom_dve(
                            RECIPROCAL_APPROX_FAST, out=stg[:], in0=dstg[:],
                            s0=cst["s0"], s1=cst["s1"], imm2=cst["imm2"])
                        # lhsT [1,128] = [ones|zeros] (A) / [zeros|ones] (B):
                        # rank-1 broadcast lands on that head's partition rows;
                        # the two accumulate into one PSUM tile
                        nc.tensor.matmul(
                            rbT[:], onr[:, hb * P:(hb + 1) * P], stg[:],
                            start=(hb == 0), stop=(hb == 1))
                    nc.scalar.copy(rT[:], rbT[:])
                    for hb, ov_tile in ((0, ovA), (1, ovB)):
                        rows = slice(0, DV) if hb == 0 else slice(DV, P)
                        nc.vector.tensor_mul(
                            oT[mi][rows, c * CH:(c + 1) * CH],
                            ov_tile[rows, :], rT[rows, :])
                return run

            def unit_oproj(c, st):
                def run():
                    ob = obp.tile([P, D], BF16, name="ob", tag="ob")
                    for nh in range(2):
                        pq = pp_ps.tile([P, CH], F32, name="po", tag="pp")
                        for mi in range(NM):
                            nc.tensor.matmul(
                                pq[:], oT[mi][:, st * P:(st + 1) * P],
                                wo_t[mi][:, nh * CH:(nh + 1) * CH],
                                start=(mi == 0), stop=(mi == NM - 1))
                        nc.vector.tensor_copy(ob[:, nh * CH:(nh + 1) * CH], pq[:])
                    eng = nc.sync if st % 2 == 0 else nc.gpsimd
                    eng.dma_start(out[st * P:(st + 1) * P, :], ob[:])
                return run

            # ---- attention ----
            def attention_chain(c, mi):
                nt = 4 * c + 4
                ovA = ov_ps.tile([P, CH], F32, name="ovA", tag="ov")
                ovB = ov_ps.tile([P, CH], F32, name="ovB", tag="ov")
                for t in range(nt):
                    drain(lambda k, b=t // 4: k[0] == "kv" and k[1] <= b)
                    # region: one k-tile x both heads; double-buffered so the
                    # next step's scores never wait on this step's exp
                    reg = reg_ps.tile([P, 2 * CH], F32, name="reg", tag="reg")
                    for hb in range(2):  # A/B back-to-back -> concurrent rows
                        nc.tensor.matmul(
                            reg[:, hb * CH:(hb + 1) * CH],
                            kT[mi][hb * DK:(hb + 1) * DK, t * P:(t + 1) * P],
                            qT[mi][hb * DK:(hb + 1) * DK, c * CH:(c + 1) * CH],
                            start=True, stop=True)
                    pump(1)
                    ex = ep.tile([P, 2 * CH], BF16, name="ex", tag="ex")
                    nc.scalar.activation(ex[:], reg[:], Exp)
                    r = t - 4 * c
                    lo = max(r, 0) * P
                    if r >= 0:  # diagonal-block masks (DVE)
                        for hb in range(2):
                            co = hb * CH + lo
                            nc.vector.tensor_mul(
                                ex[:, co:co + P], ex[:, co:co + P], mA[:])
                    # attn@v (+denominator); masked column prefixes zero-filled
                    # (partial-column PSUM accumulation mis-executes on HW)
                    if lo > 0:
                        for hb in range(2):
                            co = hb * CH
                            nc.vector.tensor_copy(ex[:, co:co + lo], zb[:, 0:lo])
                    for hb, ov in ((0, ovA), (1, ovB)):
                        nc.tensor.matmul(
                            ov[:],
                            vaug[t][:, mi * 2 * P + hb * P:mi * 2 * P + (hb + 1) * P],
                            ex[:, hb * CH:(hb + 1) * CH],
                            start=(t == 0), stop=(t == nt - 1))
                    pump(1)
                # emitted directly: the next chain's attn@v recycles these ov
                # slots, so their normalize must precede it in every stream
                unit_norm(c, mi, ovA, ovB)()

            # ---- main pipeline ----
            enqueue_block(0)
            drain(lambda k: k[0] in ("q", "kv") and k[1] == 0)
            for c in range(NCH):
                if c + 1 < NCH:
                    enqueue_block(c + 1)
                drain(lambda k, c=c: k[0] == "q" and k[1] <= c)
                for mi in range(NM):
                    attention_chain(c, mi)
                for st in range(4 * c, 4 * c + 4):
                    fq.append((("no", c), unit_oproj(c, st)))
            while fq:
                fq.popleft()[1]()
    nc.compile()
    return nc


_NC_CACHE = {}
LAST_RESULT = None


def _get_nc():
    if "nc" not in _NC_CACHE:
        import concourse.tile as tile
        import concourse.mybir as mybir
        from concourse import bacc
        nc = bacc.Bacc("TRN2", target_bir_lowering=False, num_devices=NCORES)
        _NC_CACHE["nc"] = build(nc, tile, mybir)
    return _NC_CACHE["nc"]


def onesr_pat():
    o = np.zeros((1, 2 * P), dtype=np.float32)
    o[0, 0:DV] = 1.0           # A: rows 0-63
    o[0, P + DV:2 * P] = 1.0   # B: rows 64-127
    return o


def make_consts():
    import ml_dtypes
    i = np.arange(P)[:, None]
    j = np.arange(P)[None, :]
    maskA = (j >= i).astype(ml_dtypes.bfloat16)
    vinit = np.zeros((P, 4 * P), dtype=np.float32)
    for mi in range(NM):
        vinit[:, mi * 2 * P + DV] = 1.0  # A ones col -> den row 64
        vinit[:, mi * 2 * P + P] = 1.0   # B ones col -> den row 0
    return maskA, vinit.astype(ml_dtypes.bfloat16)


def kernel(Q, K, V, Wq, Wk, Wv, Wo):
    import ml_dtypes
    from concourse.bass_utils import run_bass_kernel_spmd

    BF = ml_dtypes.bfloat16
    Q = np.asarray(Q, dtype=np.float32)
    K = np.asarray(K, dtype=np.float32)
    V = np.asarray(V, dtype=np.float32)
    Wq = np.asarray(Wq, dtype=np.float32) * np.float32(1.0 / np.sqrt(DK))
    Wk = np.asarray(Wk, dtype=np.float32)
    Wv = np.asarray(Wv, dtype=np.float32)
    Wo = np.asarray(Wo, dtype=np.float32)

    QT = [np.ascontiguousarray(Q[b].T).astype(BF) for b in range(B)]
    KT = [np.ascontiguousarray(K[b].T).astype(BF) for b in range(B)]
    VT = [np.ascontiguousarray(V[b].T).astype(BF) for b in range(B)]
    maskA, vinit = make_consts()

    in_maps = []
    for core in range(NCORES):
        b, g = core // HG, core % HG
        cs = slice(g * HDC, (g + 1) * HDC)
        wv_c = Wv[:, cs].reshape(D, HPC, DK)
        # head order [h0, h2, h1, h3]: A-dims then B-dims per pair
        wv_c = wv_c[:, [0, 2, 1, 3], :].reshape(D, HDC)
        in_maps.append({
            "xqT": QT[b], "xkT": KT[b], "xvT": VT[b],
            "wqkv": np.ascontiguousarray(
                np.concatenate([Wq[:, cs], Wk[:, cs], wv_c], axis=1)).astype(BF),
            "wod": np.ascontiguousarray(Wo[cs, :]).astype(BF),
            "maskA": maskA, "vinit": vinit,
            "onesr": onesr_pat(),
        })

    nc = _get_nc()
    res = run_bass_kernel_spmd(nc, in_maps, core_ids=list(range(NCORES)))
    global LAST_RESULT
    LAST_RESULT = res

    acc = np.zeros((B, S, D), dtype=np.float64)
    for core in range(NCORES):
        acc[core // HG] += res.results[core]["out"].astype(np.float64)
    return acc.astype(np.float32)


# revision 33
# speedup vs baseline: 1.4458x; 1.0459x over previous
"""Multi-head causal attention (B=2, S=2048, D=1024, H=16, DK=DV=64) on 8 Trainium2
NeuronCores.

Sharding: 2-way batch x 4-way head-group. Core i handles batch i//4 and heads
[4*(i%4), 4*(i%4)+4). Each core projects q/k/v for its head group, runs causal
attention, and computes a partial output projection through its row-block of Wo.
The 4 partial outputs per batch are summed on the host.

v2 design (vs the f32r baseline):
- Everything bf16 (inputs downcast on host): full-rate PE, FWL weight loads,
  half the HBM traffic. PSUM accumulation stays fp32.
- Scores for a head PAIR run concurrently on the PE via row-group tiling
  (K=64 each; heads 2mi/2mi+1 live on partitions 0-63/64-127 of qT/kT).
- v is projected in natural [seq, dv] layout (no PE transposes) and scattered
  into padded per-k-tile lhsT blocks: A = [dims(64)|ones|zeros(63)],
  B = [ones|zeros(63)|dims(64)]. attn@v with M=128 then lands head A's
  numerator on PSUM partitions 0-63 (denominator row 64) and head B's on
  64-127 (denominator row 0), so the softmax normalize multiply is a single
  partition-aligned DVE op per head fused with the PSUM->SBUF cast.
- Scores for 2 k-tiles x 2 heads accumulate into one [128, 2048] PSUM region
  (4 banks); exp runs as two [128,1024] ACT instructions (A-half / B-half) so
  the next step's A-scores only wait on the A-half exp (subtile deps).
- Denominator reciprocal via the fast approx DVE op straight out of PSUM;
  broadcast across 64 partitions with a gpsimd partition_broadcast.
- Causal masking: diagonal 128-blocks get a triangular mask multiply on
  gpsimd (SBUF bf16); fully-masked column prefixes are simply skipped by
  restricting the attn@v matmul to [lo:512].
- The whole kernel is software-pipelined with an explicit filler queue:
  projection / normalize / output-projection work is emitted between the
  attention steps so the PE never idles behind the ACT exp chain.
"""
import sys

sys.path.insert(0, "/opt/trn_rl_repo")
import numpy as np

B, S, D = 2, 2048, 1024
H, DK, DV = 16, 64, 64
NCORES = 8
HG = 4          # head-group cores per batch
HPC = 4         # heads per core
HDC = HPC * DK  # 256 projection cols per core
P = 128         # partitions
CH = 512        # q-chunk size
NCH = S // CH   # 4 q-chunks (also 512-col seq blocks)
NST = S // P    # 16 s-tiles (k-tiles)
ND = D // P     # 8 d-tiles
NM = 2          # head pairs


def build(nc, tile, mybir):
    from collections import deque
    from contextlib import ExitStack

    BF16 = mybir.dt.bfloat16
    F32 = mybir.dt.float32
    Exp = mybir.ActivationFunctionType.Exp

    xqT = nc.dram_tensor("xqT", [D, S], BF16, kind="ExternalInput").ap()
    xkT = nc.dram_tensor("xkT", [D, S], BF16, kind="ExternalInput").ap()
    xvT = nc.dram_tensor("xvT", [D, S], BF16, kind="ExternalInput").ap()
    wqkv = nc.dram_tensor("wqkv", [D, 3 * HDC], BF16, kind="ExternalInput").ap()
    wod = nc.dram_tensor("wod", [HDC, D], BF16, kind="ExternalInput").ap()
    maskA = nc.dram_tensor("maskA", [P, P], BF16, kind="ExternalInput").ap()
    vinit = nc.dram_tensor("vinit", [P, 4 * P], BF16, kind="ExternalInput").ap()
    onesr = nc.dram_tensor("onesr", [1, 2 * P], mybir.dt.float32r,
                           kind="ExternalInput").ap()
    out = nc.dram_tensor("out", [S, D], BF16, kind="ExternalOutput").ap()

    with tile.TileContext(nc) as tc:
        with ExitStack() as ctx:
            wp = ctx.enter_context(tc.tile_pool(name="wp", bufs=1))
            ep = ctx.enter_context(tc.tile_pool(name="ep", bufs=3))
            obp = ctx.enter_context(tc.tile_pool(name="obp", bufs=2))
            sp = ctx.enter_context(tc.tile_pool(name="sp", bufs=4))
            rp = ctx.enter_context(tc.tile_pool(name="rp", bufs=2))
            pp_ps = ctx.enter_context(tc.tile_pool(name="pp_ps", bufs=2, space="PSUM"))
            reg_ps = ctx.enter_context(tc.tile_pool(name="reg_ps", bufs=2, space="PSUM"))
            ov_ps = ctx.enter_context(tc.tile_pool(name="ov_ps", bufs=2, space="PSUM"))

            # ---- persistent SBUF tiles ----
            xq_t = [wp.tile([P, S], BF16, name=f"xq{i}") for i in range(ND)]
            xk_t = [wp.tile([P, S], BF16, name=f"xk{i}") for i in range(ND)]
            xv_t = [wp.tile([P, S], BF16, name=f"xv{i}") for i in range(ND)]
            wqkv_t = [wp.tile([P, 3 * HDC], BF16, name=f"wqkv{i}") for i in range(ND)]
            wq_t = [wqkv_t[i][:, 0:HDC] for i in range(ND)]
            wk_t = [wqkv_t[i][:, HDC:2 * HDC] for i in range(ND)]
            wv_t = [wqkv_t[i][:, 2 * HDC:3 * HDC] for i in range(ND)]
            wo_t = [wp.tile([P, D], BF16, name=f"wo{i}") for i in range(NM)]
            mA = wp.tile([P, P], BF16, name="mA")
            onr = wp.tile([1, 2 * P], mybir.dt.float32r, name="onr")
            nc.sync.dma_start(onr[:], onesr[:, :])
            zb = wp.tile([P, 3 * P], BF16, name="zb")
            nc.vector.memset(zb[:], 0.0)
            qT = [wp.tile([P, S], BF16, name=f"qT{m}") for m in range(NM)]
            kT = [wp.tile([P, S], BF16, name=f"kT{m}") for m in range(NM)]
            oT = [wp.tile([P, S], BF16, name=f"oT{m}") for m in range(NM)]
            vaug = [wp.tile([P, 4 * P], BF16, name=f"vaug{t}") for t in range(NST)]

            # ---- DMAs: weights + vaug init, then x (block-0 quarter first) ----
            for i in range(ND):
                nc.sync.dma_start(wqkv_t[i][:], wqkv[i * P:(i + 1) * P, :])
            for i in range(NM):
                nc.sync.dma_start(wo_t[i][:], wod[i * P:(i + 1) * P, :])
            nc.sync.dma_start(mA[:], maskA[:, :])
            for t in range(NST):
                eng = nc.gpsimd if t % 2 == 0 else nc.scalar
                eng.dma_start(vaug[t][:], vinit[:, :])
            dq = [nc.sync, nc.gpsimd]
            n = 0
            for cols in (slice(0, 2 * CH), slice(2 * CH, S)):
                for xs, xd in ((xk_t, xkT), (xq_t, xqT), (xv_t, xvT)):
                    for dd in range(ND):
                        dq[n % 2].dma_start(xs[dd][:, cols],
                                            xd[dd * P:(dd + 1) * P, cols])
                        n += 1

            # ---- filler queue ----
            fq = deque()

            def pump(k):
                for _ in range(k):
                    if not fq:
                        return
                    fq.popleft()[1]()

            def drain(match):
                if not any(match(key) for key, _ in fq):
                    return
                while fq:
                    key, fn = fq.popleft()
                    fn()
                    if not any(match(k2) for k2, _ in fq):
                        return

            # ---- projection units ----
            def unit_kq(w_views, dstT, mi, xs, sb):
                def run():
                    pq = pp_ps.tile([P, CH], F32, name="pq", tag="pp")
                    for dd in range(ND):
                        nc.tensor.matmul(
                            pq[:], w_views[dd][:, mi * P:(mi + 1) * P],
                            xs[dd][:, sb * CH:(sb + 1) * CH],
                            start=(dd == 0), stop=(dd == ND - 1))
                    nc.vector.tensor_copy(dstT[mi][:, sb * CH:(sb + 1) * CH], pq[:])
                return run

            def unit_v(st):
                # natural-layout v for s-tile st: [128 seq, 256 dv] then scatter
                # into vaug[st]: A-dims -> [256mi, 256mi+64), B-dims -> [256mi+192, ...)
                def run():
                    pv = pp_ps.tile([P, HDC], F32, name="pv", tag="pp")
                    for dd in range(ND):
                        nc.tensor.matmul(
                            pv[:], xv_t[dd][:, st * P:(st + 1) * P], wv_t[dd][:],
                            start=(dd == 0), stop=(dd == ND - 1))
                    dstA = vaug[st].rearrange("p (mi x) -> p mi x", mi=2)[:, :, 0:DV]
                    srcA = pv[:, 0:P].rearrange("p (mi d) -> p mi d", mi=2)
                    nc.vector.tensor_copy(dstA, srcA)
                    dstB = vaug[st].rearrange("p (mi x) -> p mi x", mi=2)[:, :, 3 * DK:4 * DK]
                    srcB = pv[:, P:2 * P].rearrange("p (mi d) -> p mi d", mi=2)
                    nc.vector.tensor_copy(dstB, srcB)
                return run

            def enqueue_block(sb):
                for mi in range(NM):
                    fq.append((("q", sb), unit_kq(wq_t, qT, mi, xq_t, sb)))
                for mi in range(NM):
                    fq.append((("kv", sb), unit_kq(wk_t, kT, mi, xk_t, sb)))
                for st in range(4 * sb, 4 * sb + 4):
                    fq.append((("kv", sb), unit_v(st)))

            # ---- normalize + output projection units ----
            def unit_norm(c, mi, ovA, ovB):
                # head A: dims rows 0-63, den row 64; head B: dims 64-127, den 0.
                # rbT = ones(64x1) @ recip(den) rank-1 broadcasts per head into
                # one PSUM tile (A rows 0-63 at col-group 0, B at col-group 64,
                # concurrent); copied to SBUF once, then one fused
                # normalize-multiply per head out of ov PSUM.
                def run():
                    rT = rp.tile([P, CH], BF16, name="rT", tag="rT")
                    rbT = pp_ps.tile([P, CH], F32, name="rbT", tag="pp")
                    for hb, ov_tile, dr in ((0, ovA, DV), (1, ovB, 0)):
                        dstg = sp.tile([1, CH], F32, name="dstg", tag="dstg")
                        stg = sp.tile([1, CH], mybir.dt.float32r,
                                      name="stg", tag="stg")
                        nc.vector.tensor_copy(dstg[:], ov_tile[dr:dr + 1, :])
                        # reciprocal_approx_fast with f32r out (same bit
                        # layout; the helper's f32-only assert is over-strict)
                        from concourse.dve_ops import (
                            RECIP_APPROX_FAST_CONSTS, RECIPROCAL_APPROX_FAST)
                        cst = RECIP_APPROX_FAST_CONSTS
                        nc.vector._custom_dve(
                            RECIPROCAL_APPROX_FAST, out=stg[:], in0=dstg[:],
                            s0=cst["s0"], s1=cst["s1"], imm2=cst["imm2"])
                        # lhsT [1,128] = [ones|zeros] (A) / [zeros|ones] (B):
                        # rank-1 broadcast lands on that head's partition rows;
                        # the two accumulate into one PSUM tile
                        nc.tensor.matmul(
                            rbT[:], onr[:, hb * P:(hb + 1) * P], stg[:],
                            start=(hb == 0), stop=(hb == 1))
                    nc.vector.tensor_copy(rT[:], rbT[:])
                    for hb, ov_tile in ((0, ovA), (1, ovB)):
                        rows = slice(0, DV) if hb == 0 else slice(DV, P)
                        nc.vector.tensor_mul(
                            oT[mi][rows, c * CH:(c + 1) * CH],
                            ov_tile[rows, :], rT[rows, :])
                return run

            def unit_oproj(c, st):
                def run():
                    ob = obp.tile([P, D], BF16, name="ob", tag="ob")
                    for nh in range(2):
                        pq = pp_ps.tile([P, CH], F32, name="po", tag="pp")
                        for mi in range(NM):
                            nc.tensor.matmul(
                                pq[:], oT[mi][:, st * P:(st + 1) * P],
                                wo_t[mi][:, nh * CH:(nh + 1) * CH],
                                start=(mi == 0), stop=(mi == NM - 1))
                        nc.vector.tensor_copy(ob[:, nh * CH:(nh + 1) * CH], pq[:])
                    eng = nc.sync if st % 2 == 0 else nc.gpsimd
                    eng.dma_start(out[st * P:(st + 1) * P, :], ob[:])
                return run

            # ---- attention ----
            def attention_chain(c, mi):
                nt = 4 * c + 4
                ovA = ov_ps.tile([P, CH], F32, name="ovA", tag="ov")
                ovB = ov_ps.tile([P, CH], F32, name="ovB", tag="ov")

                def scores(t):
                    # one k-tile x both heads; double-buffered region so these
                    # never wait on the previous step's exp
                    drain(lambda k, b=t // 4: k[0] == "kv" and k[1] <= b)
                    reg = reg_ps.tile([P, 2 * CH], F32, name="reg", tag="reg")
                    for hb in range(2):  # A/B back-to-back -> concurrent rows
                        nc.tensor.matmul(
                            reg[:, hb * CH:(hb + 1) * CH],
                            kT[mi][hb * DK:(hb + 1) * DK, t * P:(t + 1) * P],
                            qT[mi][hb * DK:(hb + 1) * DK, c * CH:(c + 1) * CH],
                            start=True, stop=True)
                    return reg

                # pipelined: exp(t) issues while scores(t+1) run, so the ACT
                # exp stream stays back-to-back
                reg = scores(0)
                for t in range(nt):
                    ex = ep.tile([P, 2 * CH], BF16, name="ex", tag="ex")
                    nc.scalar.activation(ex[:], reg[:], Exp)
                    if t + 1 < nt:
                        reg = scores(t + 1)
                    r = t - 4 * c
                    lo = max(r, 0) * P
                    if r >= 0:  # diagonal-block masks (DVE)
                        for hb in range(2):
                            co = hb * CH + lo
                            nc.vector.tensor_mul(
                                ex[:, co:co + P], ex[:, co:co + P], mA[:])
                    # attn@v (+denominator); masked column prefixes zero-filled
                    # (partial-column PSUM accumulation mis-executes on HW)
                    if lo > 0:
                        for hb in range(2):
                            co = hb * CH
                            nc.vector.tensor_copy(ex[:, co:co + lo], zb[:, 0:lo])
                    for hb, ov in ((0, ovA), (1, ovB)):
                        nc.tensor.matmul(
                            ov[:],
                            vaug[t][:, mi * 2 * P + hb * P:mi * 2 * P + (hb + 1) * P],
                            ex[:, hb * CH:(hb + 1) * CH],
                            start=(t == 0), stop=(t == nt - 1))
                    pump(1)
                # emitted directly: the next chain's attn@v recycles these ov
                # slots, so their normalize must precede it in every stream
                unit_norm(c, mi, ovA, ovB)()

            # ---- main pipeline ----
            enqueue_block(0)
            drain(lambda k: k[0] in ("q", "kv") and k[1] == 0)
            for c in range(NCH):
                if c + 1 < NCH:
                    enqueue_block(c + 1)
                drain(lambda k, c=c: k[0] == "q" and k[1] <= c)
                for mi in range(NM):
                    attention_chain(c, mi)
                for st in range(4 * c, 4 * c + 4):
                    fq.append((("no", c), unit_oproj(c, st)))
            while fq:
                fq.popleft()[1]()
    nc.compile()
    return nc


_NC_CACHE = {}
LAST_RESULT = None


def _get_nc():
    if "nc" not in _NC_CACHE:
        import concourse.tile as tile
        import concourse.mybir as mybir
        from concourse import bacc
        nc = bacc.Bacc("TRN2", target_bir_lowering=False, num_devices=NCORES)
        _NC_CACHE["nc"] = build(nc, tile, mybir)
    return _NC_CACHE["nc"]


def onesr_pat():
    o = np.zeros((1, 2 * P), dtype=np.float32)
    o[0, 0:DV] = 1.0           # A: rows 0-63
    o[0, P + DV:2 * P] = 1.0   # B: rows 64-127
    return o


def make_consts():
    import ml_dtypes
    i = np.arange(P)[:, None]
    j = np.arange(P)[None, :]
    maskA = (j >= i).astype(ml_dtypes.bfloat16)
    vinit = np.zeros((P, 4 * P), dtype=np.float32)
    for mi in range(NM):
        vinit[:, mi * 2 * P + DV] = 1.0  # A ones col -> den row 64
        vinit[:, mi * 2 * P + P] = 1.0   # B ones col -> den row 0
    return maskA, vinit.astype(ml_dtypes.bfloat16)


def kernel(Q, K, V, Wq, Wk, Wv, Wo):
    import ml_dtypes
    from concourse.bass_utils import run_bass_kernel_spmd

    BF = ml_dtypes.bfloat16
    Q = np.asarray(Q, dtype=np.float32)
    K = np.asarray(K, dtype=np.float32)
    V = np.asarray(V, dtype=np.float32)
    Wq = np.asarray(Wq, dtype=np.float32) * np.float32(1.0 / np.sqrt(DK))
    Wk = np.asarray(Wk, dtype=np.float32)
    Wv = np.asarray(Wv, dtype=np.float32)
    Wo = np.asarray(Wo, dtype=np.float32)

    QT = [np.ascontiguousarray(Q[b].T).astype(BF) for b in range(B)]
    KT = [np.ascontiguousarray(K[b].T).astype(BF) for b in range(B)]
    VT = [np.ascontiguousarray(V[b].T).astype(BF) for b in range(B)]
    maskA, vinit = make_consts()

    in_maps = []
    for core in range(NCORES):
        b, g = core // HG, core % HG
        cs = slice(g * HDC, (g + 1) * HDC)
        wv_c = Wv[:, cs].reshape(D, HPC, DK)
        # head order [h0, h2, h1, h3]: A-dims then B-dims per pair
        wv_c = wv_c[:, [0, 2, 1, 3], :].reshape(D, HDC)
        in_maps.append({
            "xqT": QT[b], "xkT": KT[b], "xvT": VT[b],
            "wqkv": np.ascontiguousarray(
                np.concatenate([Wq[:, cs], Wk[:, cs], wv_c], axis=1)).astype(BF),
            "wod": np.ascontiguousarray(Wo[cs, :]).astype(BF),
            "maskA": maskA, "vinit": vinit,
            "onesr": onesr_pat(),
        })

    nc = _get_nc()
    res = run_bass_kernel_spmd(nc, in_maps, core_ids=list(range(NCORES)))
    global LAST_RESULT
    LAST_RESULT = res

    acc = np.zeros((B, S, D), dtype=np.float64)
    for core in range(NCORES):
        acc[core // HG] += res.results[core]["out"].astype(np.float64)
    return acc.astype(np.float32)


# revision 39
# speedup vs baseline: 1.4894x; 1.0302x over previous
"""Multi-head causal attention (B=2, S=2048, D=1024, H=16, DK=DV=64) on 8 Trainium2
NeuronCores.

Sharding: 2-way batch x 4-way head-group. Core i handles batch i//4 and heads
[4*(i%4), 4*(i%4)+4). Each core projects q/k/v for its head group, runs causal
attention, and computes a partial output projection through its row-block of Wo.
The 4 partial outputs per batch are summed on the host.

v2 design (vs the f32r baseline):
- Everything bf16 (inputs downcast on host): full-rate PE, FWL weight loads,
  half the HBM traffic. PSUM accumulation stays fp32.
- Scores for a head PAIR run concurrently on the PE via row-group tiling
  (K=64 each; heads 2mi/2mi+1 live on partitions 0-63/64-127 of qT/kT).
- v is projected in natural [seq, dv] layout (no PE transposes) and scattered
  into padded per-k-tile lhsT blocks: A = [dims(64)|ones|zeros(63)],
  B = [ones|zeros(63)|dims(64)]. attn@v with M=128 then lands head A's
  numerator on PSUM partitions 0-63 (denominator row 64) and head B's on
  64-127 (denominator row 0), so the softmax normalize multiply is a single
  partition-aligned DVE op per head fused with the PSUM->SBUF cast.
- Scores for 2 k-tiles x 2 heads accumulate into one [128, 2048] PSUM region
  (4 banks); exp runs as two [128,1024] ACT instructions (A-half / B-half) so
  the next step's A-scores only wait on the A-half exp (subtile deps).
- Denominator reciprocal via the fast approx DVE op straight out of PSUM;
  broadcast across 64 partitions with a gpsimd partition_broadcast.
- Causal masking: diagonal 128-blocks get a triangular mask multiply on
  gpsimd (SBUF bf16); fully-masked column prefixes are simply skipped by
  restricting the attn@v matmul to [lo:512].
- The whole kernel is software-pipelined with an explicit filler queue:
  projection / normalize / output-projection work is emitted between the
  attention steps so the PE never idles behind the ACT exp chain.
"""
import sys

sys.path.insert(0, "/opt/trn_rl_repo")
import numpy as np

B, S, D = 2, 2048, 1024
H, DK, DV = 16, 64, 64
NCORES = 8
HG = 4          # head-group cores per batch
HPC = 4         # heads per core
HDC = HPC * DK  # 256 projection cols per core
P = 128         # partitions
CH = 512        # q-chunk size
NCH = S // CH   # 4 q-chunks (also 512-col seq blocks)
NST = S // P    # 16 s-tiles (k-tiles)
ND = D // P     # 8 d-tiles
NM = 2          # head pairs


def build(nc, tile, mybir):
    from collections import deque
    from contextlib import ExitStack

    BF16 = mybir.dt.bfloat16
    F32 = mybir.dt.float32
    Exp = mybir.ActivationFunctionType.Exp

    xqT = nc.dram_tensor("xqT", [D, S], BF16, kind="ExternalInput").ap()
    xkT = nc.dram_tensor("xkT", [D, S], BF16, kind="ExternalInput").ap()
    xvT = nc.dram_tensor("xvT", [D, S], BF16, kind="ExternalInput").ap()
    wqkv = nc.dram_tensor("wqkv", [D, 3 * HDC], BF16, kind="ExternalInput").ap()
    wod = nc.dram_tensor("wod", [HDC, D], BF16, kind="ExternalInput").ap()
    maskA = nc.dram_tensor("maskA", [P, P], BF16, kind="ExternalInput").ap()
    vinit = nc.dram_tensor("vinit", [P, 4 * P], BF16, kind="ExternalInput").ap()
    onesr = nc.dram_tensor("onesr", [1, 2 * P], mybir.dt.float32r,
                           kind="ExternalInput").ap()
    out = nc.dram_tensor("out", [S, D], BF16, kind="ExternalOutput").ap()

    with tile.TileContext(nc) as tc:
        with ExitStack() as ctx:
            wp = ctx.enter_context(tc.tile_pool(name="wp", bufs=1))
            ep = ctx.enter_context(tc.tile_pool(name="ep", bufs=3))
            obp = ctx.enter_context(tc.tile_pool(name="obp", bufs=2))
            sp = ctx.enter_context(tc.tile_pool(name="sp", bufs=4))
            rp = ctx.enter_context(tc.tile_pool(name="rp", bufs=2))
            pp_ps = ctx.enter_context(tc.tile_pool(name="pp_ps", bufs=2, space="PSUM"))
            reg_ps = ctx.enter_context(tc.tile_pool(name="reg_ps", bufs=2, space="PSUM"))
            ov_ps = ctx.enter_context(tc.tile_pool(name="ov_ps", bufs=2, space="PSUM"))

            # ---- persistent SBUF tiles ----
            xq_t = [wp.tile([P, S], BF16, name=f"xq{i}") for i in range(ND)]
            xk_t = [wp.tile([P, S], BF16, name=f"xk{i}") for i in range(ND)]
            xv_t = [wp.tile([P, S], BF16, name=f"xv{i}") for i in range(ND)]
            wqkv_t = [wp.tile([P, 3 * HDC], BF16, name=f"wqkv{i}") for i in range(ND)]
            wq_t = [wqkv_t[i][:, 0:HDC] for i in range(ND)]
            wk_t = [wqkv_t[i][:, HDC:2 * HDC] for i in range(ND)]
            wv_t = [wqkv_t[i][:, 2 * HDC:3 * HDC] for i in range(ND)]
            wo_t = [wp.tile([P, D], BF16, name=f"wo{i}") for i in range(NM)]
            mA = wp.tile([P, P], BF16, name="mA")
            onr = wp.tile([1, 2 * P], mybir.dt.float32r, name="onr")
            nc.sync.dma_start(onr[:], onesr[:, :])
            zb = wp.tile([P, 3 * P], BF16, name="zb")
            nc.vector.memset(zb[:], 0.0)
            qT = [wp.tile([P, S], BF16, name=f"qT{m}") for m in range(NM)]
            kT = [wp.tile([P, S], BF16, name=f"kT{m}") for m in range(NM)]
            oT = [wp.tile([P, S], BF16, name=f"oT{m}") for m in range(NM)]
            vaug = [wp.tile([P, 4 * P], BF16, name=f"vaug{t}") for t in range(NST)]

            # ---- DMAs: weights + vaug init, then x (block-0 quarter first) ----
            for i in range(ND):
                nc.sync.dma_start(wqkv_t[i][:], wqkv[i * P:(i + 1) * P, :])
            for i in range(NM):
                nc.sync.dma_start(wo_t[i][:], wod[i * P:(i + 1) * P, :])
            nc.sync.dma_start(mA[:], maskA[:, :])
            for t in range(NST):
                eng = nc.gpsimd if t % 2 == 0 else nc.scalar
                eng.dma_start(vaug[t][:], vinit[:, :])
            dq = [nc.sync, nc.gpsimd]
            n = 0
            for cols in (slice(0, CH), slice(CH, 2 * CH), slice(2 * CH, S)):
                for xs, xd in ((xk_t, xkT), (xq_t, xqT), (xv_t, xvT)):
                    for dd in range(ND):
                        dq[n % 2].dma_start(xs[dd][:, cols],
                                            xd[dd * P:(dd + 1) * P, cols])
                        n += 1

            # ---- filler queue ----
            fq = deque()

            def pump(k):
                for _ in range(k):
                    if not fq:
                        return
                    fq.popleft()[1]()

            def drain(match):
                if not any(match(key) for key, _ in fq):
                    return
                while fq:
                    key, fn = fq.popleft()
                    fn()
                    if not any(match(k2) for k2, _ in fq):
                        return

            # ---- projection units ----
            def unit_kq(w_views, dstT, mi, xs, sb):
                def run():
                    pq = pp_ps.tile([P, CH], F32, name="pq", tag="pp")
                    for dd in range(ND):
                        nc.tensor.matmul(
                            pq[:], w_views[dd][:, mi * P:(mi + 1) * P],
                            xs[dd][:, sb * CH:(sb + 1) * CH],
                            start=(dd == 0), stop=(dd == ND - 1))
                    nc.vector.tensor_copy(dstT[mi][:, sb * CH:(sb + 1) * CH], pq[:])
                return run

            def unit_v(st):
                # natural-layout v for s-tile st: [128 seq, 256 dv] then scatter
                # into vaug[st]: A-dims -> [256mi, 256mi+64), B-dims -> [256mi+192, ...)
                def run():
                    pv = pp_ps.tile([P, HDC], F32, name="pv", tag="pp")
                    for dd in range(ND):
                        nc.tensor.matmul(
                            pv[:], xv_t[dd][:, st * P:(st + 1) * P], wv_t[dd][:],
                            start=(dd == 0), stop=(dd == ND - 1))
                    dstA = vaug[st].rearrange("p (mi x) -> p mi x", mi=2)[:, :, 0:DV]
                    srcA = pv[:, 0:P].rearrange("p (mi d) -> p mi d", mi=2)
                    nc.vector.tensor_copy(dstA, srcA)
                    dstB = vaug[st].rearrange("p (mi x) -> p mi x", mi=2)[:, :, 3 * DK:4 * DK]
                    srcB = pv[:, P:2 * P].rearrange("p (mi d) -> p mi d", mi=2)
                    nc.vector.tensor_copy(dstB, srcB)
                return run

            def enqueue_block(sb):
                fq.append((("q", sb, 0), unit_kq(wq_t, qT, 0, xq_t, sb)))
                fq.append((("kv", sb), unit_kq(wk_t, kT, 0, xk_t, sb)))
                for st in range(4 * sb, 4 * sb + 4):
                    fq.append((("kv", sb), unit_v(st)))
                fq.append((("q", sb, 1), unit_kq(wq_t, qT, 1, xq_t, sb)))
                fq.append((("kv", sb), unit_kq(wk_t, kT, 1, xk_t, sb)))

            # ---- normalize + output projection units ----
            def unit_norm(c, mi, ovA, ovB):
                # head A: dims rows 0-63, den row 64; head B: dims 64-127, den 0.
                # rbT = ones(64x1) @ recip(den) rank-1 broadcasts per head into
                # one PSUM tile (A rows 0-63 at col-group 0, B at col-group 64,
                # concurrent); copied to SBUF once, then one fused
                # normalize-multiply per head out of ov PSUM.
                def run():
                    ctx2 = tc.high_priority(offset=300)
                    ctx2.__enter__()
                    rT = rp.tile([P, CH], BF16, name="rT", tag="rT")
                    rbT = pp_ps.tile([P, CH], F32, name="rbT", tag="pp")
                    for hb, ov_tile, dr in ((0, ovA, DV), (1, ovB, 0)):
                        dstg = sp.tile([1, CH], F32, name="dstg", tag="dstg")
                        stg = sp.tile([1, CH], mybir.dt.float32r,
                                      name="stg", tag="stg")
                        nc.vector.tensor_copy(dstg[:], ov_tile[dr:dr + 1, :])
                        # reciprocal_approx_fast with f32r out (same bit
                        # layout; the helper's f32-only assert is over-strict)
                        from concourse.dve_ops import (
                            RECIP_APPROX_FAST_CONSTS, RECIPROCAL_APPROX_FAST)
                        cst = RECIP_APPROX_FAST_CONSTS
                        nc.vector._custom_dve(
                            RECIPROCAL_APPROX_FAST, out=stg[:], in0=dstg[:],
                            s0=cst["s0"], s1=cst["s1"], imm2=cst["imm2"])
                        # lhsT [1,128] = [ones|zeros] (A) / [zeros|ones] (B):
                        # rank-1 broadcast lands on that head's partition rows;
                        # the two accumulate into one PSUM tile
                        nc.tensor.matmul(
                            rbT[:], onr[:, hb * P:(hb + 1) * P], stg[:],
                            start=(hb == 0), stop=(hb == 1))
                    nc.vector.tensor_copy(rT[:], rbT[:])
                    for hb, ov_tile in ((0, ovA), (1, ovB)):
                        rows = slice(0, DV) if hb == 0 else slice(DV, P)
                        nc.vector.tensor_mul(
                            oT[mi][rows, c * CH:(c + 1) * CH],
                            ov_tile[rows, :], rT[rows, :])
                    ctx2.__exit__(None, None, None)
                return run

            def unit_oproj(c, st):
                def run():
                    ob = obp.tile([P, D], BF16, name="ob", tag="ob")
                    for nh in range(2):
                        pq = pp_ps.tile([P, CH], F32, name="po", tag="pp")
                        for mi in range(NM):
                            nc.tensor.matmul(
                                pq[:], oT[mi][:, st * P:(st + 1) * P],
                                wo_t[mi][:, nh * CH:(nh + 1) * CH],
                                start=(mi == 0), stop=(mi == NM - 1))
                        nc.vector.tensor_copy(ob[:, nh * CH:(nh + 1) * CH], pq[:])
                    eng = nc.sync if st % 2 == 0 else nc.gpsimd
                    eng.dma_start(out[st * P:(st + 1) * P, :], ob[:])
                return run

            # ---- attention ----
            def attention_chain(c, mi):
                nt = 4 * c + 4
                ovA = ov_ps.tile([P, CH], F32, name="ovA", tag="ov")
                ovB = ov_ps.tile([P, CH], F32, name="ovB", tag="ov")

                drain(lambda k, c=c, mi=mi: k[0] == "q" and
                      (k[1] < c or (k[1] == c and k[2] == mi)))

                def scores(t):
                    # one k-tile x both heads; double-buffered region so these
                    # never wait on the previous step's exp
                    drain(lambda k, b=t // 4: k[0] == "kv" and k[1] <= b)
                    reg = reg_ps.tile([P, 2 * CH], F32, name="reg", tag="reg")
                    for hb in range(2):  # A/B back-to-back -> concurrent rows
                        nc.tensor.matmul(
                            reg[:, hb * CH:(hb + 1) * CH],
                            kT[mi][hb * DK:(hb + 1) * DK, t * P:(t + 1) * P],
                            qT[mi][hb * DK:(hb + 1) * DK, c * CH:(c + 1) * CH],
                            start=True, stop=True)
                    return reg

                # pipelined: exp(t) issues while scores(t+1) run, so the ACT
                # exp stream stays back-to-back
                reg = scores(0)
                for t in range(nt):
                    ex = ep.tile([P, 2 * CH], BF16, name="ex", tag="ex")
                    nc.scalar.activation(ex[:], reg[:], Exp)
                    if t + 1 < nt:
                        reg = scores(t + 1)
                    r = t - 4 * c
                    lo = max(r, 0) * P
                    if r >= 0:  # diagonal-block masks (DVE)
                        for hb in range(2):
                            co = hb * CH + lo
                            nc.vector.tensor_mul(
                                ex[:, co:co + P], ex[:, co:co + P], mA[:])
                    # attn@v (+denominator); masked column prefixes zero-filled
                    # (partial-column PSUM accumulation mis-executes on HW)
                    if lo > 0:
                        for hb in range(2):
                            co = hb * CH
                            nc.vector.tensor_copy(ex[:, co:co + lo], zb[:, 0:lo])
                    for hb, ov in ((0, ovA), (1, ovB)):
                        nc.tensor.matmul(
                            ov[:],
                            vaug[t][:, mi * 2 * P + hb * P:mi * 2 * P + (hb + 1) * P],
                            ex[:, hb * CH:(hb + 1) * CH],
                            start=(t == 0), stop=(t == nt - 1))
                    pump(1)
                # emitted directly: the next chain's attn@v recycles these ov
                # slots, so their normalize must precede it in every stream
                unit_norm(c, mi, ovA, ovB)()

            # ---- main pipeline ----
            enqueue_block(0)
            for c in range(NCH):
                if c + 1 < NCH:
                    enqueue_block(c + 1)
                for mi in range(NM):
                    attention_chain(c, mi)
                for st in range(4 * c, 4 * c + 4):
                    fq.append((("no", c), unit_oproj(c, st)))
            while fq:
                fq.popleft()[1]()
    nc.compile()
    return nc


_NC_CACHE = {}
LAST_RESULT = None


def _get_nc():
    if "nc" not in _NC_CACHE:
        import concourse.tile as tile
        import concourse.mybir as mybir
        from concourse import bacc
        nc = bacc.Bacc("TRN2", target_bir_lowering=False, num_devices=NCORES)
        _NC_CACHE["nc"] = build(nc, tile, mybir)
    return _NC_CACHE["nc"]


def onesr_pat():
    o = np.zeros((1, 2 * P), dtype=np.float32)
    o[0, 0:DV] = 1.0           # A: rows 0-63
    o[0, P + DV:2 * P] = 1.0   # B: rows 64-127
    return o


def make_consts():
    import ml_dtypes
    i = np.arange(P)[:, None]
    j = np.arange(P)[None, :]
    maskA = (j >= i).astype(ml_dtypes.bfloat16)
    vinit = np.zeros((P, 4 * P), dtype=np.float32)
    for mi in range(NM):
        vinit[:, mi * 2 * P + DV] = 1.0  # A ones col -> den row 64
        vinit[:, mi * 2 * P + P] = 1.0   # B ones col -> den row 0
    return maskA, vinit.astype(ml_dtypes.bfloat16)


def kernel(Q, K, V, Wq, Wk, Wv, Wo):
    import ml_dtypes
    from concourse.bass_utils import run_bass_kernel_spmd

    BF = ml_dtypes.bfloat16
    Q = np.asarray(Q, dtype=np.float32)
    K = np.asarray(K, dtype=np.float32)
    V = np.asarray(V, dtype=np.float32)
    Wq = np.asarray(Wq, dtype=np.float32) * np.float32(1.0 / np.sqrt(DK))
    Wk = np.asarray(Wk, dtype=np.float32)
    Wv = np.asarray(Wv, dtype=np.float32)
    Wo = np.asarray(Wo, dtype=np.float32)

    QT = [np.ascontiguousarray(Q[b].T).astype(BF) for b in range(B)]
    KT = [np.ascontiguousarray(K[b].T).astype(BF) for b in range(B)]
    VT = [np.ascontiguousarray(V[b].T).astype(BF) for b in range(B)]
    maskA, vinit = make_consts()

    in_maps = []
    for core in range(NCORES):
        b, g = core // HG, core % HG
        cs = slice(g * HDC, (g + 1) * HDC)
        wv_c = Wv[:, cs].reshape(D, HPC, DK)
        # head order [h0, h2, h1, h3]: A-dims then B-dims per pair
        wv_c = wv_c[:, [0, 2, 1, 3], :].reshape(D, HDC)
        in_maps.append({
            "xqT": QT[b], "xkT": KT[b], "xvT": VT[b],
            "wqkv": np.ascontiguousarray(
                np.concatenate([Wq[:, cs], Wk[:, cs], wv_c], axis=1)).astype(BF),
            "wod": np.ascontiguousarray(Wo[cs, :]).astype(BF),
            "maskA": maskA, "vinit": vinit,
            "onesr": onesr_pat(),
        })

    nc = _get_nc()
    res = run_bass_kernel_spmd(nc, in_maps, core_ids=list(range(NCORES)))
    global LAST_RESULT
    LAST_RESULT = res

    acc = np.zeros((B, S, D), dtype=np.float64)
    for core in range(NCORES):
        acc[core // HG] += res.results[core]["out"].astype(np.float64)
    return acc.astype(np.float32)
